# revision 20
# baseline (speedup 1.0000x reference)
"""Multi-head self-attention on 8 Trainium2 NeuronCores.

Problem: B=2, L=2048, E=1024, H=16 heads, D=64 (fp32 in/out).
Sharding: 2-way batch x 4-way head-group. Core c handles batch c//4 and
heads 4*(c%4) .. 4*(c%4)+3 (a 256-wide slice of the QKV output dim).
Each core computes a partial output y_c = Attn_c @ W_O[slice]; the host
sums the 4 partials per batch (the "all-reduce" of row-parallel W_O).

Perf design (v7) - the kernel is PE-stream-bound (bf16 floor ~164us);
the schedule aims PE busy from ~1us after the framework preamble to
the last output chunk with no head/tail idle:
 - Input DMA is strictly need-ordered and compute-paced. Each x chunk
   is DMA'd as two 512-col halves on the sync/gpsimd HW queues (each
   queue FIFO => chunk-granular in-order delivery at full bandwidth).
   lh0 K/Q projections run chunk-outer (4 accumulating matmuls per
   chunk into 4 psum banks) so the PE consumes each chunk ~0.85us
   after it lands. V chunks are issued from the DVE stream after the
   Q bias-adds (so they don't steal bandwidth from K/Q), lh1 chunks
   are paced by x-pool WAR rotation behind V.
 - Stage order is qt-outermost: (qt, h, gi) with 8 k-groups of 2
   chunks. After a qt column finishes (all 4 heads + normalize), its
   output projection (16 matmuls) + psum copies + output DMA ride
   under the next qt's stages, so output work and the 4MB output DMA
   are spread through the whole attention phase instead of a tail.
   lh1 K/Q projection rides under qt1's first half; V projection +
   PE transposes ride under qt0.
 - PSUM budget 8 banks: scores 2x2 (GRP=2), PV accumulators 2, and 2
   for whichever rider is active (head proj / V proj / lh1 proj /
   out-proj) - pools are opened/closed LIFO around each phase.
 - Scores are computed transposed, St = [k, q]; softmax denominator
   comes free as row 64 of the PV psum via a ones column in V (V tiles
   padded to 128 cols so PV LDWEIGHTS can use fast-weight-load).
 - exp on ScalarE with the 1/sqrt(D) scale folded in; no max
   subtraction (logits bounded ~|4|, exp can't overflow). The ScalarE
   runs ONLY the 128 exp calls (~124us): copies go to DVE/gpsimd and
   no DMA is ever issued from the scalar queue after startup.
 - PV lags exp (et pool sized for the backlog) and catches up once the
   V tiles are transposed; all matmul operands bf16.
 - B_V is folded on the host: softmax rows sum to 1, so the V bias adds
   the constant row B_V @ W_O to the output.
"""

import sys

if "/opt/trn_rl_repo" not in sys.path:
    sys.path.insert(0, "/opt/trn_rl_repo")

import numpy as np
import ml_dtypes

B, L, E = 2, 2048, 1024
H, D = 16, 64
OC = 256          # per-core slice of the H*D output dim (4 heads)
HC = OC // D      # heads per core = 4
ECH = E // 128    # 8 e-chunks
LT = L // 512     # 4 l-tiles of 512
KC = L // 128     # 16 k-chunks
NG = 8            # k-groups per (qt, h) stage column, 2 chunks each
N_WARM = 6        # warm-up matmuls at t=0 (HAM release + DMA-ramp cover)
ET_BUFS = 30      # et pool depth = max scores-ahead-of-PV backlog + slack
PV_START = 19       # first stage index allowed to emit PV (V ready)

_CACHE = {}


def _build():
    from concourse import bacc, tile, mybir
    from concourse import masks

    f32 = mybir.dt.float32
    bf16 = mybir.dt.bfloat16
    Exp = mybir.ActivationFunctionType.Exp

    nc = bacc.Bacc("TRN2", target_bir_lowering=False, debug=False)

    # partition-major packed inputs: [p, half, c, l] with e-chunk = c*2+half
    qT = nc.dram_tensor("qT", [128, 2, 4, L], bf16, kind="ExternalInput").ap()
    kT = nc.dram_tensor("kT", [128, 2, 4, L], bf16, kind="ExternalInput").ap()
    vT = nc.dram_tensor("vT", [128, 2, 4, L], bf16, kind="ExternalInput").ap()
    wq = nc.dram_tensor("wq", [128, ECH, OC], bf16, kind="ExternalInput").ap()
    wk = nc.dram_tensor("wk", [128, ECH, OC], bf16, kind="ExternalInput").ap()
    wv = nc.dram_tensor("wv", [128, ECH, OC], bf16, kind="ExternalInput").ap()
    wo = nc.dram_tensor("wo", [128, 2, E], bf16, kind="ExternalInput").ap()
    bq = nc.dram_tensor("bq", [128, 2, 1], f32, kind="ExternalInput").ap()
    bk = nc.dram_tensor("bk", [128, 2, 1], f32, kind="ExternalInput").ap()
    yT = nc.dram_tensor("yT", [E, L], bf16, kind="ExternalOutput").ap()

    with tile.TileContext(nc) as tc:
        with (
            tc.tile_pool(name="w", bufs=1) as wp,
            tc.tile_pool(name="xt", bufs=16) as xp,
            tc.tile_pool(name="qk", bufs=1) as qkp,
            tc.tile_pool(name="vt", bufs=1) as vtp,
            tc.tile_pool(name="et", bufs=ET_BUFS) as ep,
            tc.tile_pool(name="norm", bufs=2) as npl,
            tc.tile_pool(name="yst", bufs=4) as ysp,
        ):
            # ---- PE warm-up: matmuls on (mostly) uninitialized SBUF -
            # the result is never read; a 1-column memset allocates the
            # tile without gating the matmuls on a full-tile write ----
            warm = wp.tile([128, 512], bf16, tag="warm")
            nc.vector.memset(warm[:], 0.0)
            with tc.tile_pool(name="ps_wu", bufs=1, space="PSUM") as pwu:
                pw = pwu.tile([128, 512], f32, tag="pw")
                for i in range(N_WARM):
                    nc.tensor.matmul(pw[:], warm[:, 0:128], warm[:],
                                     start=True, stop=True)

            # ---- weights + biases. K first (needed first); each weight
            # DMA'd as two e-halves split across the sync/gpsimd queues
            # so it streams at full bandwidth ahead of its x chunks ----
            twk = wp.tile([128, ECH, OC], bf16, tag="twk")
            twq = wp.tile([128, ECH, OC], bf16, tag="twq")
            twv = wp.tile([128, ECH, OC], bf16, tag="twv")
            two = wp.tile([128, 2, E], bf16, tag="two")
            tbq = wp.tile([128, 2, 1], f32, tag="tbq")
            tbk = wp.tile([128, 2, 1], f32, tag="tbk")
            nc.sync.dma_start(twk[:, 0:4, :], wk[:, 0:4, :])
            nc.gpsimd.dma_start(twk[:, 4:8, :], wk[:, 4:8, :])
            nc.gpsimd.dma_start(tbk[:], bk)
            nc.gpsimd.dma_start(tbq[:], bq)

            # ---- persistent activations ----
            qt_t = [[qkp.tile([128, 1024], bf16, tag=f"qt{m}_{lh}",
                              name=f"qt{m}_{lh}") for lh in range(2)]
                    for m in range(2)]
            kt_t = [[qkp.tile([128, 1024], bf16, tag=f"kt{m}_{lh}",
                              name=f"kt{m}_{lh}") for lh in range(2)]
                    for m in range(2)]
            ot_t = [qkp.tile([128, L], bf16, tag=f"ot{m}", name=f"ot{m}")
                    for m in range(2)]
            # V with a ones column per head: one tile per l-tile of 4
            # k-chunks, [l, kc4, h, 128] (padded to 128 cols for FWL)
            v_t4 = [vtp.tile([128, 4, HC, 128], bf16, tag=f"v{i}", name=f"v{i}")
                    for i in range(LT)]
            for i in range(LT):
                nc.vector.memset(v_t4[i][:, :, :, D:], 0.0)
                nc.vector.memset(v_t4[i][:, :, :, D:D + 1], 1.0)

            # ---- x chunk streaming: [128, 1024] tiles; each DMA'd as
            # two 512-col halves (sync queue gets half 0, gpsimd half 1)
            # so per-queue FIFO order = need order at full bandwidth.
            # One 16-buf pool for all 48 tiles: V tiles reuse the k-lh0
            # bufs, lh1 tiles reuse V bufs -> WAR rotation paces the
            # later DMAs behind the compute that consumes the earlier
            # ones. V chunk DMAs are additionally issued from the DVE
            # stream (after the Q bias-adds) so they can't steal
            # bandwidth from the critical K/Q-lh0 window. ----
            xch = {}
            x_rr = [0]

            def x_chunks(name, src, lh):
                for half in range(2):
                    for c in range(4):
                        x = xp.tile([128, 1024], bf16, tag="x",
                                    name=f"x{name}_{half}_{c}_{lh}")
                        s = src[:, half, c, lh * 1024:(lh + 1) * 1024]
                        eng = (nc.sync, nc.gpsimd)[x_rr[0] % 2]
                        x_rr[0] += 1
                        eng.dma_start(x[:], s)
                        xch[(name, half, c, lh)] = x

            x_chunks("k", kT, 0)
            x_chunks("k", kT, 1)
            nc.sync.dma_start(twq[:, 0:4, :], wq[:, 0:4, :])
            nc.gpsimd.dma_start(twq[:, 4:8, :], wq[:, 4:8, :])
            x_chunks("q", qT, 0)
            nc.sync.dma_start(twv[:, 0:4, :], wv[:, 0:4, :])
            nc.gpsimd.dma_start(twv[:, 4:8, :], wv[:, 4:8, :])

            # ---- head projection (all of K, plus Q l-half 0),
            # chunk-outer: per chunk, 4 accumulating matmuls (m x lt)
            # into 4 psum banks, so each chunk is consumed (and its buf
            # freed) right after landing. Bias-adds on DVE. ----
            def proj_chunk_outer(name, wt, tb, dst, lh):
                pp = {}
                for m in range(2):
                    for lt2 in range(2):
                        pp[(m, lt2)] = pskq.tile(
                            [128, 512], f32, tag="pp",
                            name=f"pp{name}{lh}_{m}_{lt2}")
                for half in range(2):
                    for c in range(4):
                        e = c * 2 + half
                        x = xch[(name, half, c, lh)]
                        for m in range(2):
                            for lt2 in range(2):
                                nc.tensor.matmul(
                                    pp[(m, lt2)][:],
                                    wt[:, e, m * 128:(m + 1) * 128],
                                    x[:, lt2 * 512:(lt2 + 1) * 512],
                                    start=(e == 0), stop=(e == ECH - 1),
                                    skip_group_check=True)
                for m in range(2):
                    for lt2 in range(2):
                        lt = lh * 2 + lt2
                        nc.vector.tensor_scalar_add(
                            dst[m][lh][:, lt2 * 512:(lt2 + 1) * 512],
                            pp[(m, lt2)][:], tb[:, m, :])

            pskq_cm = tc.tile_pool(name="ps_kq", bufs=8, space="PSUM")
            pskq = pskq_cm.__enter__()
            proj_chunk_outer("k", twk, tbk, kt_t, 0)
            proj_chunk_outer("k", twk, tbk, kt_t, 1)
            proj_chunk_outer("q", twq, tbq, qt_t, 0)
            pskq_cm.__exit__(None, None, None)

            # V chunks reuse the k/q bufs (emitted after the head
            # projection so the pool WAR rotation holds each V DMA back
            # until the matching K/Q chunk was consumed); q-lh1 chunks
            # reuse the V bufs and are emitted from the stage loop after
            # their V-projection readers. The queues stay need-ordered
            # end to end: K, q-lh0, V, q-lh1, W_O.
            x_chunks("v", vT, 0)
            x_chunks("v", vT, 1)

            ident = wp.tile([128, 128], bf16, tag="ident")
            masks.make_identity(nc, ident[:])
            vt_sb = [[qkp.tile([128, 512], bf16, tag=f"vtsb{m}_{lt}",
                               name=f"vtsb{m}_{lt}") for lt in range(LT)]
                     for m in range(2)]

            # ================= attention stage machinery ==================
            # qt-outermost: all 4 heads of a q column finish together so
            # its out-projection can ride under the next column.
            stages = [(qt, h, gi)
                      for qt in range(LT) for h in range(HC)
                      for gi in range(NG)]
            NS = len(stages)

            st_t = [None] * NS
            et_t = [None] * NS

            def emit_scores(s):
                qt, h, gi = stages[s]
                m, po = h // 2, (h % 2) * 64
                st = pst.tile([128, 2, 512], f32, tag="st", name=f"st{s}")
                st_t[s] = st
                for j in range(2):
                    kc = 2 * gi + j
                    nc.tensor.matmul(
                        st[:, j, :],
                        kt_t[m][kc // 8][po:po + 64,
                                         (kc % 8) * 128:(kc % 8 + 1) * 128],
                        qt_t[m][qt // 2][po:po + 64,
                                         (qt % 2) * 512:(qt % 2 + 1) * 512],
                        start=True, stop=True)

            def emit_exp(s):
                st = st_t[s]
                et = ep.tile([128, 2, 512], bf16, tag="et", name=f"et{s}")
                et_t[s] = et
                nc.scalar.activation(et[:], st[:], Exp, scale=0.125)

            po_t = {}

            def emit_pv(s, pso):
                qt, h, gi = stages[s]
                qs = slice(qt * 512, (qt + 1) * 512)
                m, po = h // 2, (h % 2) * 64
                et = et_t[s]
                et_t[s] = None
                if (h, qt) not in po_t:
                    po_t[(h, qt)] = pso.tile([128, 512], f32, tag="po",
                                             name=f"po{h}_{qt}")
                p_o = po_t[(h, qt)]
                for j in range(2):
                    kc = 2 * gi + j
                    nc.tensor.matmul(
                        p_o[:], v_t4[kc // 4][:, kc % 4, h, :], et[:, j, :],
                        start=(kc == 0), stop=(kc == KC - 1))
                if gi == NG - 1:
                    # normalize: row 64 of p_o holds the denominators
                    # (copy to SBUF first: approx recip does bitwise ops,
                    #  which are not valid on the PSUM fp32 read path)
                    den = npl.tile([1, 512], f32, tag="den", name=f"den{s}")
                    nc.vector.tensor_copy(den[:], p_o[64:65, :])
                    rec = npl.tile([1, 512], f32, tag="rec", name=f"rec{s}")
                    nc.vector.reciprocal_approx_fast(rec[:], den[:])
                    rec_b = npl.tile([64, 512], f32, tag="recb",
                                     name=f"recb{s}")
                    nc.gpsimd.partition_broadcast(rec_b[:], rec[:])
                    nc.vector.tensor_mul(
                        ot_t[m][po:po + 64, qs], p_o[0:64, :], rec_b[:])

            # ---- ride-under work items ----
            def v_group(m, lt):
                pv = psv.tile([128, 512], f32, tag="pv", name=f"pv{m}_{lt}")
                lh, sl = lt // 2, (lt % 2)
                for e in range(ECH):
                    nc.tensor.matmul(
                        pv[:], twv[:, e, m * 128:(m + 1) * 128],
                        xch[("v", e % 2, e // 2, lh)][:, sl * 512:(sl + 1) * 512],
                        start=(e == 0), stop=(e == ECH - 1))
                nc.vector.tensor_copy(vt_sb[m][lt][:], pv[:])

            def transpose_batch(lt):
                # 8 PE transposes (4 kc x 2 m) into one borrowed psum bank,
                # then 2 wide DVE copies into the PV stationary layout.
                ptf = psv.tile([128, 512], f32, tag="pv", name=f"ptb{lt}")
                ptb = ptf[:].bitcast(bf16).rearrange("p (c m o) -> p c m o",
                                                     c=4, m=2)
                for c in range(4):
                    for m in range(2):
                        nc.tensor.transpose(
                            ptb[:, c, m, :],
                            vt_sb[m][lt][:, c * 128:(c + 1) * 128],
                            ident[:])
                src5 = ptf[:].bitcast(bf16).rearrange(
                    "p (c m h d) -> p c m h d", c=4, m=2, h=2)
                for m in range(2):
                    nc.vector.tensor_copy(
                        v_t4[lt][:, :, 2 * m:2 * m + 2, 0:D], src5[:, :, m, :, :])

            def proj_group(name, wt, tb, dst, m, lt):
                pp = pskq2.tile([128, 512], f32, tag="pp",
                                name=f"pp{name}_{m}_{lt}")
                lh, sl = lt // 2, (lt % 2)
                for half in range(2):
                    for c in range(4):
                        e = c * 2 + half
                        nc.tensor.matmul(
                            pp[:], wt[:, e, m * 128:(m + 1) * 128],
                            xch[(name, half, c, lh)][:, sl * 512:(sl + 1) * 512],
                            start=(half == 0 and c == 0),
                            stop=(half == 1 and c == 3))
                nc.vector.tensor_scalar_add(
                    dst[m][lh][:, sl * 512:(sl + 1) * 512], pp[:], tb[:, m, :])

            out_rr = [0]
            out_engs = [nc.sync, nc.gpsimd]

            def out_proj_ec(ec, lt):
                # y[ec-block, lt-block] = W_O[:, ec].T @ ot  (2 matmuls)
                py = psy.tile([128, 512], f32, tag="y", name=f"py{ec}_{lt}")
                for m in range(2):
                    nc.tensor.matmul(
                        py[:], two[:, m, ec * 128:(ec + 1) * 128],
                        ot_t[m][:, lt * 512:(lt + 1) * 512],
                        start=(m == 0), stop=(m == 1))
                ty = ysp.tile([128, 512], bf16, tag="ty", name=f"ty{ec}_{lt}")
                if lt == 3 and ec % 2:
                    # tail: the exp stream is done, ScalarE is free -
                    # alternating engines halves the copy drain time
                    nc.scalar.copy(ty[:], py[:])
                else:
                    nc.vector.tensor_copy(ty[:], py[:])
                eng = out_engs[out_rr[0] % 3]
                out_rr[0] += 1
                eng.dma_start(yT[ec * 128:(ec + 1) * 128,
                                 lt * 512:(lt + 1) * 512], ty[:])

            # ---- ride-work schedule: stage idx -> list of thunks ----
            ride = {s: [] for s in range(NS)}
            # qt0 stages 5..16: V projection + transposes
            vwork = []
            for lt in range(LT):
                vwork.append(lambda lt=lt: v_group(0, lt))
                vwork.append(lambda lt=lt: v_group(1, lt))
                vwork.append(lambda lt=lt: transpose_batch(lt))
            for i, wkr in enumerate(vwork):
                ride[5 + i].append(wkr)
            # q-lh1 x-chunk DMAs: emitted after their bufs' V readers so
            # the WAR rotation paces them behind the V stream.
            ride[10].append(lambda: x_chunks("q", qT, 1))

            def wo_dma():
                nc.sync.dma_start(two[:, 0, :], wo[:, 0, :])
                nc.gpsimd.dma_start(two[:, 1, :], wo[:, 1, :])
            ride[16].append(wo_dma)
            # qt1 first half (stages 32..47): q-lh1 projection
            fill1 = []
            for m in range(2):
                for lt in (2, 3):
                    fill1.append(("q", twq, tbq, qt_t, m, lt))
            for i, args in enumerate(fill1):
                ride[32 + 2 * i].append(lambda a=args: proj_group(*a))
            # out-proj: lt0 under qt1 second half, lt1 under qt2,
            # lt2 under qt3 (spread), lt3 at the tail.
            # (a ride at stage r sees pv_done = r-3 in the sustained
            # regime, so lt's riders start after pv stage 32*lt+31+3)
            for ec in range(ECH):
                ride[48 + 2 * ec].append(lambda ec=ec: out_proj_ec(ec, 0))
                ride[68 + 3 * ec].append(lambda ec=ec: out_proj_ec(ec, 1))
                ride[99 + 3 * ec].append(lambda ec=ec: out_proj_ec(ec, 2))

            # ---- pools for the stage phase (LIFO nesting) ----
            pst_cm = tc.tile_pool(name="ps_st", bufs=2, space="PSUM")
            pst = pst_cm.__enter__()
            psv_cm = tc.tile_pool(name="ps_v", bufs=2, space="PSUM")
            psv = psv_cm.__enter__()

            pso = None
            pso_cm = None
            pskq2 = None
            pskq2_cm = None
            psy = None
            psy_cm = None

            pv_done = 0
            for s in range(NS):
                emit_scores(s)
                emit_exp(s)
                for wkr in ride[s]:
                    wkr()
                if s == 16:          # V proj + transposes done
                    psv_cm.__exit__(None, None, None)
                    pso_cm = tc.tile_pool(name="ps_o", bufs=2, space="PSUM")
                    pso = pso_cm.__enter__()
                    pskq2_cm = tc.tile_pool(name="ps_kq2", bufs=2,
                                            space="PSUM")
                    pskq2 = pskq2_cm.__enter__()
                if s == 47:          # lh1 projection done
                    pskq2_cm.__exit__(None, None, None)
                    psy_cm = tc.tile_pool(name="ps_y", bufs=2, space="PSUM")
                    psy = psy_cm.__enter__()
                if s >= PV_START:
                    # catch up the PV backlog at ~2 stages per stage
                    target = min(s - 2, 2 * (s - PV_START + 1))
                    while pv_done < min(target, s + 1):
                        emit_pv(pv_done, pso)
                        pv_done += 1
            while pv_done < NS:
                emit_pv(pv_done, pso)
                pv_done += 1
            # tail: out-projection for the last l-tile
            for ec in range(ECH):
                out_proj_ec(ec, 3)

            psy_cm.__exit__(None, None, None)
            pso_cm.__exit__(None, None, None)
            pst_cm.__exit__(None, None, None)

    nc.compile()
    return nc


def _get_nc():
    if "nc" not in _CACHE:
        _CACHE["nc"] = _build()
    return _CACHE["nc"]


def _pack_x(xb):
    """[L, E] fp32 -> [128, 2, 4, L] bf16 with [p, h, c, l] = x[l, (c*2+h)*128+p]."""
    bf = ml_dtypes.bfloat16
    xT = xb.T.reshape(4, 2, 128, L)           # [c, h, p, l]
    return np.ascontiguousarray(xT.transpose(2, 1, 0, 3)).astype(bf)


def _make_in_maps(inputs):
    bf = ml_dtypes.bfloat16
    q = np.asarray(inputs["query"], dtype=np.float32)
    k = np.asarray(inputs["key"], dtype=np.float32)
    v = np.asarray(inputs["value"], dtype=np.float32)
    WQ = np.asarray(inputs["W_Query"], dtype=np.float32)
    WK = np.asarray(inputs["W_Key"], dtype=np.float32)
    WV = np.asarray(inputs["W_Value"], dtype=np.float32)
    WO = np.asarray(inputs["W_Output"], dtype=np.float32)
    BQ = np.asarray(inputs["B_Query"], dtype=np.float32)
    BK = np.asarray(inputs["B_Key"], dtype=np.float32)

    qp = [_pack_x(q[b]) for b in range(B)]
    kp = [_pack_x(k[b]) for b in range(B)]
    vp = [_pack_x(v[b]) for b in range(B)]

    in_maps = []
    for c in range(8):
        b, g = c // 4, c % 4
        sl = slice(OC * g, OC * (g + 1))
        # weights partition-major: [p, e, o] = W[e*128+p, o]
        wqp = np.ascontiguousarray(WQ[:, sl].reshape(ECH, 128, OC).transpose(1, 0, 2)).astype(bf)
        wkp = np.ascontiguousarray(WK[:, sl].reshape(ECH, 128, OC).transpose(1, 0, 2)).astype(bf)
        wvp = np.ascontiguousarray(WV[:, sl].reshape(ECH, 128, OC).transpose(1, 0, 2)).astype(bf)
        wop = np.ascontiguousarray(WO[sl, :].reshape(2, 128, E).transpose(1, 0, 2)).astype(bf)
        in_maps.append({
            "qT": qp[b],
            "kT": kp[b],
            "vT": vp[b],
            "wq": wqp,
            "wk": wkp,
            "wv": wvp,
            "wo": wop,
            "bq": np.ascontiguousarray(BQ[sl].reshape(2, 128, 1).transpose(1, 0, 2)),
            "bk": np.ascontiguousarray(BK[sl].reshape(2, 128, 1).transpose(1, 0, 2)),
        })
    return in_maps


def _combine(results, inputs):
    WO = np.asarray(inputs["W_Output"], dtype=np.float32)
    BV = np.asarray(inputs["B_Value"], dtype=np.float32)
    BO = np.asarray(inputs["B_Output"], dtype=np.float32)
    out = np.zeros((B, L, E), dtype=np.float32)
    for c in range(8):
        out[c // 4] += results[c]["yT"].astype(np.float32).T
    out += (BV @ WO + BO)[None, None, :]
    return out


def kernel(**inputs):
    from concourse.bass_utils import run_bass_kernel_spmd

    nc = _get_nc()
    in_maps = _make_in_maps(inputs)
    res = run_bass_kernel_spmd(nc, in_maps, list(range(8)))
    return _combine(res.results, inputs)
